# revision 1
# baseline (speedup 1.0000x reference)
"""Trainium2 Bass kernel for nn_AttentionRegression (ragged segment attention).

Math reformulation (exact):
  y[b] = g_x*f_x[b] + g_b + num[b]/den[b]
    w_t   = n_t . g_n                     (g weights applied per neighbour row)
    z_t   = exp(sigmoid(tanh(n_t @ W1n^T + f_x[seg]*w1x + b1) @ W2 + b2))
    num_b = sum_{t in seg b} z_t * w_t ;  den_b = sum z_t
  (softmax max-subtraction dropped: scores are sigmoid outputs in (0,1), so
   exp() is stable and the ratio is mathematically unchanged.)

Device layout: segments sorted by length into 16 strata; stratum k supplies one
128-segment block to each of the 8 cores, padded to the stratum max Ls[k]
(sorted contiguous strata minimize the padding, ~3%). Neighbours are shipped
transposed+bf16 as nt[128 feat, col] with col = blockbase + pos*128 +
seg_local, so the per-row scalars computed by the PE land as [seg=partition,
pos=free] and segment sums are free-dim reduces. No validity mask is shipped:
padded rows are zero so w = n.g = 0 keeps them out of the numerator, and
their shared score z0 (recomputed per block from the bias alone) is removed
from the denominator via den += (len - L)*z0.

Per 128-row tile the PE does LDWEIGHTS(nt tile) + matmul against a [128,13]
weight block (12 cols = W1n^T, col 12 = g_n). The per-sample bias fx*w1x + b1
is added on the DVE (per-block [128,12] bias tile, stride-0 broadcast over
positions) so the PE queue carries nothing but the position matmuls — the
old per-chunk rank-1 bias matmul cost ~12us of real PE time (416 cols each),
and LDWEIGHTS engine time is unmodeled in the cost model (TODO in
instruction_cost_v2.rs), so on silicon the PE is the binding engine and this
relief is a real win even though TimelineSim scores it a wash.

Modeled ~107.8us/core against a ~97us DMA floor (34.4MB bf16 @358GB/s);
the neighbour stream is gapless, alternating between the SP and gpsimd DGE
queues (dual_dma). The last block's position chunks taper down and its
softmax epilogue is split so only the final taper chunk's compute trails the
last DMA. fp8 was evaluated and rejected: e4m3 neighbours push end-to-end
rel err to 1.8e-2 against the 2e-2 gate (w_t = n.g is the critical path).
"""

import numpy as np
import ml_dtypes
from contextlib import ExitStack

import concourse.bass as bass
import concourse.bacc as bacc
import concourse.tile as tile
from concourse import mybir
from concourse.bass_utils import run_bass_kernel_spmd

B, T, NF, H = 16384, 1048576, 128, 12
NCORES = 8
SEGS_PER_BLOCK = 128
CH = 32  # positions per superchunk (psum [128, 13*CH])
F32 = mybir.dt.float32
BF16 = mybir.dt.bfloat16
AL = mybir.AluOpType
AF = mybir.ActivationFunctionType

_program_cache = {}


def build_program(Ls, nblk, nrep=1, dual_dma=False, ch=CH, bufs_big=4,
                  bufs_ps=4, bufs_hp=3, taper=(10, 6)):
    nc = bacc.Bacc(
        "TRN2",
        target_bir_lowering=False,
        debug=False,
        enable_asserts=False,
    )
    sumL = sum(Ls)
    R = 128 * sumL
    nt = nc.dram_tensor("nt", [128, R], BF16, kind="ExternalInput").ap()
    w13 = nc.dram_tensor("w13", [128, 13], BF16, kind="ExternalInput").ap()
    w2rep = nc.dram_tensor("w2rep", [128, ch * H], BF16, kind="ExternalInput").ap()
    # aux3 cols: 0 b2/2 | 1 gx | 2 gb
    aux3 = nc.dram_tensor("aux3", [128, 3], F32, kind="ExternalInput").ap()
    fxd = nc.dram_tensor("fx", [128, nblk], F32, kind="ExternalInput").ap()
    # wb cols 0:12 = w1x (bias weight on fx), 12:24 = b1; rows identical
    wbd = nc.dram_tensor("wb", [128, 2 * H], F32, kind="ExternalInput").ap()
    # nnp[p, g] = len[p,g] - Ls[g]  (minus the pad count, <= 0)
    nnpd = nc.dram_tensor("nnp", [128, nblk], F32, kind="ExternalInput").ap()
    yd = nc.dram_tensor("y", [128, nblk], F32, kind="ExternalOutput").ap()

    with tile.TileContext(nc) as tc, ExitStack() as ctx:
        if nrep > 1:
            ctx.enter_context(tc.For_i(0, nrep, 1, name="bench"))
        singles = ctx.enter_context(tc.tile_pool(name="singles", bufs=1))
        bigp = ctx.enter_context(tc.tile_pool(name="bigp", bufs=bufs_big))
        psp = ctx.enter_context(tc.tile_pool(name="psp", bufs=bufs_ps, space="PSUM"))
        hp = ctx.enter_context(tc.tile_pool(name="hp", bufs=bufs_hp))

        # small loads ride the gpsimd SWDGE queue so the SP queue can start
        # streaming neighbour blocks immediately
        w13_s = singles.tile([128, 13], BF16)
        nc.gpsimd.dma_start(out=w13_s[:], in_=w13)
        w2rep_s = singles.tile([128, ch * H], BF16)
        nc.gpsimd.dma_start(out=w2rep_s[:], in_=w2rep)
        aux3_s = singles.tile([128, 3], F32)
        nc.gpsimd.dma_start(out=aux3_s[:], in_=aux3)
        fx_s = singles.tile([128, nblk], F32)
        nc.gpsimd.dma_start(out=fx_s[:], in_=fxd)
        wb_s = singles.tile([128, 2 * H], F32)
        nc.gpsimd.dma_start(out=wb_s[:], in_=wbd)
        nnp_s = singles.tile([128, nblk], F32)
        nc.gpsimd.dma_start(out=nnp_s[:], in_=nnpd)

        s_all = singles.tile([128, sumL], F32)
        w_all = singles.tile([128, sumL], F32)
        den_all = singles.tile([128, nblk], F32)
        num_all = singles.tile([128, nblk], F32)
        den2 = singles.tile([128, 2], F32)
        num2 = singles.tile([128, 2], F32)

        def epilogue(e0, elen, dcol, ncol):
            # softmax-sum epilogue, fully inside the {Tanh, Exp, Copy} set:
            # sigmoid(x) = 0.5 + 0.5*tanh(x/2) and softmax drops constants, so
            # z = exp(0.5*tanh(0.5*(s + b2))) has the exact softmax ratios.
            # No mask: padded positions have w = n.g = 0 so the numerator is
            # unaffected, and their z contribution (npad copies of the shared
            # z0 value) is subtracted from the denominator per block.
            u = hp.tile([128, elen], F32, tag="u")
            nc.scalar.activation(out=u[:], in_=s_all[:, e0: e0 + elen],
                                 func=AF.Tanh, bias=aux3_s[:, 0:1], scale=0.5)
            z = hp.tile([128, elen], F32, tag="z")
            nc.scalar.activation(out=z[:], in_=u[:], func=AF.Exp, scale=0.5)
            zw = hp.tile([128, elen], F32, tag="zw")
            nc.vector.tensor_mul(zw[:], z[:], w_all[:, e0: e0 + elen])
            nc.vector.reduce_sum(out=dcol, in_=z[:],
                                 axis=mybir.AxisListType.X)
            nc.vector.reduce_sum(out=ncol, in_=zw[:],
                                 axis=mybir.AxisListType.X)

        def chunk_sizes(L, last_block):
            """Chunk positions; on the last block taper the final chunks so
            the post-DMA compute drain is short."""
            if not last_block:
                return [min(ch, L - p) for p in range(0, L, ch)]
            tail = []
            rem = L
            for t in taper:
                if rem - t <= 0:
                    break
                tail.append(t)
                rem -= t
            head = [min(ch, rem - p) for p in range(0, rem, ch)] if rem else []
            return head + tail

        col = 0
        soff = 0
        nchunk = 0
        for g in range(nblk):
            L = Ls[g]
            last = g == nblk - 1
            sizes = chunk_sizes(L, last)
            splitA = L - sizes[-1] if (last and len(sizes) > 1) else None
            # per-block tanh bias: bias_g[p, j] = fx[p, g]*w1x[j] + b1[j].
            # Adding it on the DVE (broadcast over positions) keeps the PE
            # queue free of the old per-chunk rank-1 bias matmul, which cost
            # ~12us of PE sequencer occupancy across the program.
            bias_g = hp.tile([128, H], F32, tag="bias")
            nc.vector.scalar_tensor_tensor(
                out=bias_g[:], in0=wb_s[:, 0:H], scalar=fx_s[:, g:g + 1],
                in1=wb_s[:, H:2 * H], op0=AL.mult, op1=AL.add)
            # padded positions share one score: s0 = sum_j W2_j tanh(bias_g_j)
            # -> z0; their denominator contribution is nnp*z0 (subtracted
            # after the block's den reduce below)
            th0 = hp.tile([128, H], BF16, tag="th0")
            nc.scalar.activation(out=th0[:], in_=bias_g[:], func=AF.Tanh)
            m0 = hp.tile([128, H], BF16, tag="m0")
            nc.vector.tensor_mul(m0[:], th0[:], w2rep_s[:, 0:H])
            s0 = hp.tile([128, 1], F32, tag="s0")
            nc.vector.reduce_sum(out=s0[:], in_=m0[:],
                                 axis=mybir.AxisListType.X)
            u0 = hp.tile([128, 1], F32, tag="u0")
            nc.scalar.activation(out=u0[:], in_=s0[:], func=AF.Tanh,
                                 bias=aux3_s[:, 0:1], scale=0.5)
            z0 = hp.tile([128, 1], F32, tag="z0")
            nc.scalar.activation(out=z0[:], in_=u0[:], func=AF.Exp, scale=0.5)
            p0 = 0
            for c in sizes:
                assert p0 + c <= L
                ntb = bigp.tile([128, 128 * c], BF16, tag="ntb")
                eng = nc.gpsimd if (dual_dma and nchunk % 2) else nc.sync
                nchunk += 1
                eng.dma_start(
                    out=ntb[:],
                    in_=nt[:, col + p0 * 128: col + (p0 + c) * 128])
                ps = psp.tile([128, 13 * c], F32, tag="ps")
                for i in range(c):
                    # each position writes a disjoint 13-col psum region, so
                    # every matmul can start=True (resets only its own region)
                    # while one stop=True closes the whole chunk's group —
                    # a single sem update instead of one per position.
                    nc.tensor.matmul(
                        ps[:, 13 * i: 13 * (i + 1)],
                        lhsT=ntb[:, i * 128: (i + 1) * 128],
                        rhs=w13_s[:], start=True, stop=(i == c - 1),
                        skip_group_check=True)
                psv = ps[:].rearrange("p (c t) -> p c t", t=13)
                ti = hp.tile([128, c * H], BF16, tag="ti")
                nc.vector.tensor_add(
                    ti[:].rearrange("p (c t) -> p c t", t=H),
                    psv[:, :, 0:12],
                    bias_g[:].unsqueeze(1).broadcast_to([128, c, H]))
                th = hp.tile([128, c * H], BF16, tag="th")
                nc.scalar.activation(
                    out=th[:].rearrange("p (c t) -> p c t", t=H),
                    in_=ti[:].rearrange("p (c t) -> p c t", t=H), func=AF.Tanh)
                m = hp.tile([128, c * H], BF16, tag="m")
                nc.vector.tensor_mul(m[:], th[:], w2rep_s[:, 0: c * H])
                nc.vector.reduce_sum(
                    out=s_all[:, soff + p0: soff + p0 + c],
                    in_=m[:].rearrange("p (c t) -> p c t", t=H),
                    axis=mybir.AxisListType.X)
                nc.scalar.activation(
                    out=w_all[:, soff + p0: soff + p0 + c],
                    in_=psv[:, :, 12], func=AF.Copy)
                p0 += c
                if splitA is not None and p0 == splitA:
                    # last block: drain most of the epilogue early so only
                    # the final taper chunk's work trails the last DMA; the
                    # pad correction rides the early partial so it stays off
                    # the drain-tail chain
                    epilogue(soff, splitA, den2[:, 0:1], num2[:, 0:1])
                    nc.vector.scalar_tensor_tensor(
                        out=den2[:, 0:1], in0=nnp_s[:, g:g + 1],
                        scalar=z0[:, 0:1], in1=den2[:, 0:1],
                        op0=AL.mult, op1=AL.add)

            if splitA is not None:
                epilogue(soff + splitA, L - splitA, den2[:, 1:2], num2[:, 1:2])
                nc.vector.reduce_sum(out=den_all[:, g:g + 1], in_=den2[:],
                                     axis=mybir.AxisListType.X)
                nc.vector.reduce_sum(out=num_all[:, g:g + 1], in_=num2[:],
                                     axis=mybir.AxisListType.X)
            else:
                epilogue(soff, L, den_all[:, g:g + 1], num_all[:, g:g + 1])
                # den += nnp*z0  (nnp = -pad count, removes padded z mass)
                nc.vector.scalar_tensor_tensor(
                    out=den_all[:, g:g + 1], in0=nnp_s[:, g:g + 1],
                    scalar=z0[:, 0:1], in1=den_all[:, g:g + 1],
                    op0=AL.mult, op1=AL.add)
            col += 128 * L
            soff += L

        den_eps = singles.tile([128, nblk], F32)
        nc.vector.tensor_scalar(
            out=den_eps[:], in0=den_all[:], scalar1=1e-30, scalar2=None,
            op0=AL.add)
        rec_all = singles.tile([128, nblk], F32)
        nc.vector.reciprocal(out=rec_all[:], in_=den_eps[:])
        t_all = singles.tile([128, nblk], F32)
        nc.vector.tensor_mul(t_all[:], num_all[:], rec_all[:])
        y1_all = singles.tile([128, nblk], F32)
        nc.vector.scalar_tensor_tensor(
            out=y1_all[:], in0=fx_s[:], scalar=aux3_s[:, 1:2], in1=t_all[:],
            op0=AL.mult, op1=AL.add)
        y_all = singles.tile([128, nblk], F32)
        nc.vector.tensor_scalar(
            out=y_all[:], in0=y1_all[:], scalar1=aux3_s[:, 2:3], scalar2=None,
            op0=AL.add)
        nc.sync.dma_start(out=yd, in_=y_all[:])
    nc.compile()
    return nc, R


def prep_host(f_x, neighbours, seg_ids, f_W1, f_b1, f_W2, f_b2, g_W, g_b):
    """Shard/pack inputs. Returns (Ls, nblk, in_maps, order)."""
    lens_all = np.bincount(seg_ids, minlength=B).astype(np.int64)
    order = np.argsort(-lens_all, kind="stable")
    nblk = B // (SEGS_PER_BLOCK * NCORES)  # 16
    stratum = SEGS_PER_BLOCK * NCORES  # 1024
    Ls = []
    for k in range(nblk):
        m = int(lens_all[order[k * stratum:(k + 1) * stratum]].max())
        Ls.append(max(1, m))
    sumL = sum(Ls)
    R = 128 * sumL

    row_start = np.zeros(B + 1, np.int64)
    row_start[1:] = np.cumsum(lens_all)
    nbf = neighbours.astype(ml_dtypes.bfloat16)

    w13 = np.zeros((128, 13), np.float32)
    w13[:, 0:12] = f_W1[:, 1:].T
    w13[:, 12] = g_W[0, 1:]
    w13 = w13.astype(ml_dtypes.bfloat16)

    w1x = f_W1[:, 0].astype(np.float32)
    w2rep = np.tile(np.concatenate([f_W2[0], ]).astype(np.float32), CH)
    w2rep = np.tile(w2rep[None, :], (128, 1)).astype(ml_dtypes.bfloat16)

    aux3 = np.zeros((128, 3), np.float32)
    aux3[:, 0] = 0.5 * f_b2[0]
    aux3[:, 1] = g_W[0, 0]
    aux3[:, 2] = g_b[0]

    wb = np.empty((128, 2 * H), np.float32)
    wb[:, 0:H] = w1x[None, :]
    wb[:, H:2 * H] = f_b1.astype(np.float32)[None, :]

    in_maps = []
    for c in range(NCORES):
        idx = np.empty(R, np.int64)
        valid = np.empty(R, bool)
        fx_mat = np.empty((128, nblk), np.float32)
        nnp = np.empty((128, nblk), np.float32)
        off = 0
        for g in range(nblk):
            Lg = Ls[g]
            gids = order[g * stratum + 128 * c: g * stratum + 128 * (c + 1)]
            pos = np.arange(Lg)[:, None]
            rows = row_start[gids][None, :] + pos          # [Lg, 128]
            val = pos < lens_all[gids][None, :]
            blockn = Lg * 128
            idx[off:off + blockn] = np.where(val, rows, 0).reshape(-1)
            valid[off:off + blockn] = val.reshape(-1)
            fx_mat[:, g] = f_x[gids, 0]
            nnp[:, g] = (lens_all[gids] - Lg).astype(np.float32)
            off += blockn
        nrows = nbf[idx]                                   # [R, 128] bf16
        nrows[~valid] = ml_dtypes.bfloat16(0)
        nt_c = np.ascontiguousarray(nrows.T)               # [128, R]
        in_maps.append({
            "nt": nt_c, "w13": w13, "w2rep": w2rep, "aux3": aux3,
            "fx": fx_mat, "wb": wb, "nnp": nnp,
        })
    return Ls, nblk, in_maps, order


def assemble_output(results, order, nblk):
    stratum = SEGS_PER_BLOCK * NCORES
    y_full = np.empty(B, np.float32)
    for c in range(NCORES):
        yc = results[c]["y"]  # [128, nblk]
        for g in range(nblk):
            y_full[order[g * stratum + 128 * c: g * stratum + 128 * (c + 1)]] = yc[:, g]
    return y_full[:, None]


def kernel(**inputs) -> np.ndarray:
    args = {k: np.asarray(v) for k, v in inputs.items()}
    Ls, nblk, in_maps, order = prep_host(
        args["f_x"], args["neighbours"], args["seg_ids"],
        args["f_W1"], args["f_b1"], args["f_W2"], args["f_b2"],
        args["g_W"], args["g_b"])
    key = (tuple(Ls), nblk)
    if key not in _program_cache:
        _program_cache[key] = build_program(Ls, nblk, dual_dma=True,
                                            bufs_big=6, bufs_ps=6, bufs_hp=4)
    nc, _ = _program_cache[key]
    res = run_bass_kernel_spmd(nc, in_maps, core_ids=list(range(NCORES)))
    return assemble_output(res.results, order, nblk)



# revision 45
# speedup vs baseline: 1288.4004x; 1288.4004x over previous
"""Trainium2 Bass kernel for nn_AttentionRegression (ragged segment attention).

Math reformulation (exact):
  y[b] = g_x*f_x[b] + g_b + num[b]/den[b]
    w_t   = n_t . g_n                     (g weights applied per neighbour row)
    z_t   = exp(sigmoid(tanh(n_t @ W1n^T + f_x[seg]*w1x + b1) @ W2 + b2))
    num_b = sum_{t in seg b} z_t * w_t ;  den_b = sum z_t
  (softmax max-subtraction dropped: scores are sigmoid outputs in (0,1), so
   exp() is stable and the ratio is mathematically unchanged.)

Precision/bandwidth split (fp8 + Householder rotation): the host applies an
orthogonal reflection H (g_n -> |g|e0) to the neighbour features and the
matching inverse to the f-net weights (W1' = W1n @ H), a pure change of input
basis. In that basis the g-projection is coordinate 0, so the per-row scalar
w_t = |g|*(Hn)_0 rides a separate bf16 plane shipped directly in the
[seg=partition, pos=free] layout the segment reduction wants, while the full
128 rotated features ship as fp8e4m3 and feed only the tanh-score matmul,
which tolerates fp8 (measured end-to-end rel err ~2.4e-3 vs the 2e-2 gate).
Per-core traffic drops from 34MB bf16 to ~17.6MB (fp8 + w plane), moving the
measured DMA floor from ~111us to ~57us at the ~305GB/s/core achieved rate.

Device layout: segments sorted by length into 16 strata; stratum k supplies
one 128-segment block to each of the 8 cores, padded to the stratum max Ls[k]
(sorted contiguous strata minimize the padding, ~3%). Neighbours are shipped
rotated+transposed+fp8 as nt[128 feat, col] with col = blockbase + pos*128 +
seg_local, so per-row scores computed by the PE land as [seg=partition,
pos=free]. No validity mask is shipped: padded rows are zero so w = 0 keeps
them out of the numerator, and their shared score z0 (recomputed per block
from the bias alone) is removed from the denominator via den += (len - L)*z0.

Per 128-row position tile the PE does LDWEIGHTS(nt tile, fp8, FWL) + a
12-col matmul against W1'^T (bf16 rhs). The neighbour stream rides the SP
HWDGE queue alone (measured faster than splitting with the gpsimd SWDGE
path) in 2MB superchunks, each feeding four 32-position psum chunks. The
per-sample tanh bias fx*w1x + b1 is a per-chunk DVE add into a per-block
tanh-input buffer (one big tanh/W2-mul/score-reduce per block instead of
per-chunk ops); all 16 blocks' bias vectors and padded-row scores z0 are
precomputed in one vectorized 7-op chain, and the padded-row denominator
correction is applied to all blocks in one op at the end. Weight/aux loads
are hoisted outside the bench loop; accumulators are double-buffered so
bench iterations pipeline like independent dispatches. The last block's
chunks taper so only a small tail of compute trails the final DMA.

Measured on trn2 (per-iteration, 8-core SPMD, paired For_i amortization):
~63us vs a ~50us DMA-only floor and ~51us compute-only floor; end-to-end
rel err 2.9e-3 against the fp32 reference (gate 2e-2).
"""

import numpy as np
import ml_dtypes
from contextlib import ExitStack

import concourse.bass as bass
import concourse.bacc as bacc
import concourse.tile as tile
from concourse import mybir
from concourse.bass_utils import run_bass_kernel_spmd

B, T, NF, H = 16384, 1048576, 128, 12
NCORES = 8
SEGS_PER_BLOCK = 128
CH = 32       # positions per psum chunk (psum [128, 12*CH])
import os as _os
DCH = int(_os.environ.get("KDCH", "128"))  # positions per DMA superchunk (2MB)
F32 = mybir.dt.float32
BF16 = mybir.dt.bfloat16
FP8 = mybir.dt.float8e4
AL = mybir.AluOpType
AF = mybir.ActivationFunctionType

_program_cache = {}


def _block_chunks(L, last, taper=(16, 8)):
    """[(dma_len, [psum_lens])] for a block of L positions."""
    tail = []
    if last:
        rem = L
        for t in taper:
            if rem - t <= 0:
                break
            tail.append((t, [t]))
            rem -= t
        L = rem
    out = []
    p = 0
    while p < L:
        d = min(DCH, L - p)
        ps = []
        q = 0
        while q < d:
            c = min(CH, d - q)
            ps.append(c)
            q += c
        out.append((d, ps))
        p += d
    return out + tail


def build_program(Ls, nblk, nrep=1, dual_dma=False, bufs_big=6,
                  bufs_ps=8, bufs_hp=6, probe=None, pe_bias=False,
                  bias_mode="dve"):
    nc = bacc.Bacc(
        "TRN2",
        target_bir_lowering=False,
        debug=False,
        enable_asserts=False,
    )
    sumL = sum(Ls)
    R = 128 * sumL
    nt = nc.dram_tensor("nt", [128, R], FP8, kind="ExternalInput").ap()
    w12 = nc.dram_tensor("w12", [128, H], BF16, kind="ExternalInput").ap()
    Lmax = max(Ls)
    w2rep = nc.dram_tensor("w2rep", [128, Lmax * H], BF16, kind="ExternalInput").ap()
    # aux3 cols: 0 b2/2 | 1 gx | 2 gb
    aux3 = nc.dram_tensor("aux3", [128, 3], F32, kind="ExternalInput").ap()
    fxd = nc.dram_tensor("fx", [128, nblk], F32, kind="ExternalInput").ap()
    # wb cols 0:12 = w1x (bias weight on fx), 12:24 = b1; rows identical
    wbd = nc.dram_tensor("wb", [128, 2 * H], F32, kind="ExternalInput").ap()
    # nnp[p, g] = len[p,g] - Ls[g]  (minus the pad count, <= 0)
    nnpd = nc.dram_tensor("nnp", [128, nblk], F32, kind="ExternalInput").ap()
    # per-row g-projection w in [seg, pos] layout, zero on padded rows
    wrd = nc.dram_tensor("wr", [128, sumL], BF16, kind="ExternalInput").ap()
    # bias-matmul operands: fxT[0, g*128+seg] = fx, fxT[1, :] = 1;
    # wbrep rows = (w1x, b1) tiled CH times
    fxTd = nc.dram_tensor("fxT", [2, 128 * nblk], BF16, kind="ExternalInput").ap()
    wbrepd = nc.dram_tensor("wbrep", [2, H * CH], BF16, kind="ExternalInput").ap()
    yd = nc.dram_tensor("y", [128, nblk], F32, kind="ExternalOutput").ap()

    with tile.TileContext(nc) as tc, ExitStack() as ctx:
        singles = ctx.enter_context(tc.tile_pool(name="singles", bufs=1))

        # loop-invariant loads: outside the bench loop so repeat timing
        # measures the steady-state body (a single dispatch pays these once,
        # overlapped with the first neighbour superchunks). They ride the
        # gpsimd SWDGE queue so the SP queue starts streaming immediately.
        w12_s = singles.tile([128, H], BF16)
        nc.gpsimd.dma_start(out=w12_s[:], in_=w12)
        w2rep_s = singles.tile([128, Lmax * H], BF16)
        nc.gpsimd.dma_start(out=w2rep_s[:], in_=w2rep)
        aux3_s = singles.tile([128, 3], F32)
        nc.gpsimd.dma_start(out=aux3_s[:], in_=aux3)
        fx_s = singles.tile([128, nblk], F32)
        nc.gpsimd.dma_start(out=fx_s[:], in_=fxd)
        wb_s = singles.tile([128, 2 * H], F32)
        nc.gpsimd.dma_start(out=wb_s[:], in_=wbd)
        nnp_s = singles.tile([128, nblk], F32)
        nc.gpsimd.dma_start(out=nnp_s[:], in_=nnpd)
        wr_s = singles.tile([128, sumL], BF16)
        nc.gpsimd.dma_start(out=wr_s[:], in_=wrd)
        fxT_s = singles.tile([2, 128 * nblk], BF16)
        nc.gpsimd.dma_start(out=fxT_s[:], in_=fxTd)
        wbrep_s = singles.tile([2, H * CH], BF16)
        nc.gpsimd.dma_start(out=wbrep_s[:], in_=wbrepd)

        if probe in ("compute", "pe"):
            # static neighbour tile reused by every matmul: removes the DMA
            # stream so the probe isolates compute-side throughput
            ntb0 = singles.tile([128, 128 * DCH], FP8)
            nc.sync.dma_start(out=ntb0[:], in_=nt[:, 0: 128 * DCH])

        if nrep > 1:
            ctx.enter_context(tc.For_i(0, nrep, 1, name="bench"))
        bigp = ctx.enter_context(tc.tile_pool(name="bigp", bufs=bufs_big))
        psp = ctx.enter_context(tc.tile_pool(name="psp", bufs=bufs_ps, space="PSUM"))
        hp = ctx.enter_context(tc.tile_pool(name="hp", bufs=bufs_hp))
        # accumulators double-buffer across bench iterations so consecutive
        # iterations pipeline like independent dispatches
        accp = ctx.enter_context(
            tc.tile_pool(name="accp", bufs=2 if nrep > 1 else 1))

        s_all = accp.tile([128, sumL], BF16, tag="s_all")
        den_all = accp.tile([128, nblk], F32, tag="den_all")
        num_all = accp.tile([128, nblk], F32, tag="num_all")
        den2 = accp.tile([128, 2], F32, tag="den2")
        num2 = accp.tile([128, 2], F32, tag="num2")

        # all 16 blocks' tanh biases + padded-row scores in one vectorized
        # chain (6 ops total instead of 6 ops per block): bias_all[p, g, j] =
        # fx[p,g]*w1x[j] + b1[j]; z0_all[p, g] = pad-row softmax weight.
        ba_t = accp.tile([128, nblk * H], F32, tag="ba_t")
        nc.vector.tensor_mul(
            ba_t[:].rearrange("p (g t) -> p g t", t=H),
            fx_s[:].unsqueeze(2).broadcast_to([128, nblk, H]),
            wb_s[:, 0:H].unsqueeze(1).broadcast_to([128, nblk, H]))
        bias_all = accp.tile([128, nblk * H], F32, tag="bias_all")
        nc.vector.tensor_add(
            bias_all[:].rearrange("p (g t) -> p g t", t=H),
            ba_t[:].rearrange("p (g t) -> p g t", t=H),
            wb_s[:, H:2 * H].unsqueeze(1).broadcast_to([128, nblk, H]))
        th0_all = accp.tile([128, nblk * H], BF16, tag="th0_all")
        nc.scalar.activation(out=th0_all[:], in_=bias_all[:], func=AF.Tanh)
        m0_all = accp.tile([128, nblk * H], BF16, tag="m0_all")
        nc.vector.tensor_mul(m0_all[:], th0_all[:], w2rep_s[:, 0: nblk * H])
        s0_all = accp.tile([128, nblk], F32, tag="s0_all")
        nc.vector.reduce_sum(out=s0_all[:],
                             in_=m0_all[:].rearrange("p (g t) -> p g t", t=H),
                             axis=mybir.AxisListType.X)
        u0_all = accp.tile([128, nblk], F32, tag="u0_all")
        nc.scalar.activation(out=u0_all[:], in_=s0_all[:], func=AF.Tanh,
                             bias=aux3_s[:, 0:1], scale=0.5)
        z0_all = accp.tile([128, nblk], F32, tag="z0_all")
        nc.scalar.activation(out=z0_all[:], in_=u0_all[:], func=AF.Exp,
                             scale=0.5)

        def epilogue(e0, elen, dcol, ncol):
            # softmax-sum epilogue, fully inside the {Tanh, Exp, Copy} set:
            # sigmoid(x) = 0.5 + 0.5*tanh(x/2) and softmax drops constants, so
            # z = exp(0.5*tanh(0.5*(s + b2))) has the exact softmax ratios.
            # No mask: padded positions have w = 0 so the numerator is
            # unaffected, and their z contribution (npad copies of the shared
            # z0 value) is subtracted from the denominator per block.
            u = hp.tile([128, elen], BF16, tag="u")
            nc.scalar.activation(out=u[:], in_=s_all[:, e0: e0 + elen],
                                 func=AF.Tanh, bias=aux3_s[:, 0:1], scale=0.5)
            z = hp.tile([128, elen], BF16, tag="z")
            nc.scalar.activation(out=z[:], in_=u[:], func=AF.Exp, scale=0.5)
            zw = hp.tile([128, elen], BF16, tag="zw")
            nc.vector.tensor_mul(zw[:], z[:], wr_s[:, e0: e0 + elen])
            nc.vector.reduce_sum(out=dcol, in_=z[:],
                                 axis=mybir.AxisListType.X)
            nc.vector.reduce_sum(out=ncol, in_=zw[:],
                                 axis=mybir.AxisListType.X)

        col = 0
        soff = 0
        ndma = 0
        for g in range(nblk):
            L = Ls[g]
            last = g == nblk - 1
            chunks = _block_chunks(L, last)
            # epilogue split point: all but the final taper chunk drain early
            splitA = L - chunks[-1][0] if (last and len(chunks) > 1) else None

            if probe == "dma":
                p0 = 0
                for d, _ in chunks:
                    ntb = bigp.tile([128, 128 * d], FP8, tag="ntb")
                    eng2 = nc.scalar if dual_dma == "act" else nc.gpsimd
                    eng = eng2 if (dual_dma and ndma % 2) else nc.sync
                    ndma += 1
                    eng.dma_start(
                        out=ntb[:],
                        in_=nt[:, col + p0 * 128: col + (p0 + d) * 128])
                    p0 += d
                col += 128 * L
                soff += L
                continue
            if probe == "pe":
                for d, psizes in chunks:
                    for c in psizes:
                        ps = psp.tile([128, H * c], F32, tag="ps")
                        if pe_bias:
                            nc.tensor.matmul(
                                ps[:], lhsT=fxT_s[:, g * 128:(g + 1) * 128],
                                rhs=wbrep_s[:, 0:H * c], start=True,
                                stop=False, skip_group_check=True)
                        for i in range(c):
                            nc.tensor.matmul(
                                ps[:, H * i: H * (i + 1)],
                                lhsT=ntb0[:, (i % DCH) * 128: (i % DCH + 1) * 128],
                                rhs=w12_s[:], start=not pe_bias,
                                stop=(i == c - 1),
                                skip_group_check=True)
                col += 128 * L
                soff += L
                continue

            bias_g = bias_all[:, g * H: (g + 1) * H]

            # per-block tanh-input (dve mode) or tanh-output (pe mode)
            # accumulator: per-chunk ops write slices, then per-span W2-mul
            # and score-reduce — few big DVE dispatches.
            tib = hp.tile([128, L * H], BF16, tag="tib")

            def post(a, b):
                """W2/score-reduce for positions [a, b) of this block."""
                n = b - a
                tv = tib[:, a * H: b * H]
                if bias_mode == "pe":
                    th = tv  # tib already holds tanh outputs (ACT from psum)
                else:
                    tht = hp.tile([128, n * H], BF16, tag="th")
                    nc.scalar.activation(
                        out=tht[:].rearrange("p (c t) -> p c t", t=H),
                        in_=tv.rearrange("p (c t) -> p c t", t=H),
                        func=AF.Tanh)
                    th = tht[:]
                m = hp.tile([128, n * H], BF16, tag="m")
                nc.vector.tensor_mul(m[:], th, w2rep_s[:, 0: n * H])
                with nc.allow_low_precision(
                        reason="12-term score sum; bf16 ample here"):
                    nc.vector.reduce_sum(
                        out=s_all[:, soff + a: soff + b],
                        in_=m[:].rearrange("p (c t) -> p c t", t=H),
                        axis=mybir.AxisListType.X)

            p0 = 0
            for d, psizes in chunks:
                if probe == "compute":
                    ntb = ntb0
                    dbase = 0
                else:
                    ntb = bigp.tile([128, 128 * d], FP8, tag="ntb")
                    eng2 = nc.scalar if dual_dma == "act" else nc.gpsimd
                    eng = eng2 if (dual_dma and ndma % 2) else nc.sync
                    ndma += 1
                    eng.dma_start(
                        out=ntb[:],
                        in_=nt[:, col + p0 * 128: col + (p0 + d) * 128])
                    dbase = 0
                for c in psizes:
                    ps = psp.tile([128, H * c], F32, tag="ps")
                    if bias_mode == "pe":
                        # rank-2 matmul seeds the chunk's psum with
                        # fx*w1x + b1; tanh then reads psum directly on the
                        # ACT engine and the DVE does no per-chunk work.
                        nc.tensor.matmul(
                            ps[:], lhsT=fxT_s[:, g * 128:(g + 1) * 128],
                            rhs=wbrep_s[:, 0:H * c], start=True, stop=False,
                            skip_group_check=True)
                    for i in range(c):
                        # each position writes a disjoint 12-col psum
                        # region; one stop=True closes the chunk's group —
                        # a single sem update per chunk.
                        nc.tensor.matmul(
                            ps[:, H * i: H * (i + 1)],
                            lhsT=ntb[:, (dbase + i) * 128: (dbase + i + 1) * 128],
                            rhs=w12_s[:], start=(bias_mode != "pe"),
                            stop=(i == c - 1),
                            skip_group_check=True)
                    psv = ps[:].rearrange("p (c t) -> p c t", t=H)
                    q0 = p0 + dbase
                    tslice = tib[:, q0 * H: (q0 + c) * H].rearrange(
                        "p (c t) -> p c t", t=H)
                    if bias_mode == "pe":
                        nc.scalar.activation(out=tslice, in_=psv,
                                             func=AF.Tanh)
                    else:
                        nc.vector.tensor_add(
                            tslice, psv,
                            bias_g.unsqueeze(1).broadcast_to([128, c, H]))
                    dbase += c
                p0 += d
                if splitA is not None and p0 == splitA:
                    # last block: drain most of the pipeline early so only
                    # the final taper chunk's work trails the last DMA
                    post(0, splitA)
                    epilogue(soff, splitA, den2[:, 0:1], num2[:, 0:1])

            if probe == "compute" and last:
                splitA = None
            if splitA is not None:
                post(splitA, L)
                epilogue(soff + splitA, L - splitA, den2[:, 1:2], num2[:, 1:2])
                nc.vector.reduce_sum(out=den_all[:, g:g + 1], in_=den2[:],
                                     axis=mybir.AxisListType.X)
                nc.vector.reduce_sum(out=num_all[:, g:g + 1], in_=num2[:],
                                     axis=mybir.AxisListType.X)
            else:
                post(0, L)
                epilogue(soff, L, den_all[:, g:g + 1], num_all[:, g:g + 1])
            col += 128 * L
            soff += L

        if probe in ("dma", "pe"):
            y_all = accp.tile([128, nblk], F32, tag="y_all")
            nc.vector.tensor_scalar(
                out=y_all[:], in0=fx_s[:], scalar1=aux3_s[:, 2:3],
                scalar2=None, op0=AL.add)
            nc.sync.dma_start(out=yd, in_=y_all[:])
        else:
            # den += nnp*z0 for all blocks at once (nnp = -pad count,
            # removes the padded rows' z mass from each denominator)
            denc = accp.tile([128, nblk], F32, tag="denc")
            nc.vector.tensor_mul(denc[:], nnp_s[:], z0_all[:])
            den_eps = accp.tile([128, nblk], F32, tag="den_eps")
            nc.vector.scalar_tensor_tensor(
                out=den_eps[:], in0=den_all[:], scalar=1e-30, in1=denc[:],
                op0=AL.add, op1=AL.add)
            rec_all = accp.tile([128, nblk], F32, tag="rec_all")
            nc.vector.reciprocal(out=rec_all[:], in_=den_eps[:])
            t_all = accp.tile([128, nblk], F32, tag="t_all")
            nc.vector.tensor_mul(t_all[:], num_all[:], rec_all[:])
            y1_all = accp.tile([128, nblk], F32, tag="y1_all")
            nc.vector.scalar_tensor_tensor(
                out=y1_all[:], in0=fx_s[:], scalar=aux3_s[:, 1:2], in1=t_all[:],
                op0=AL.mult, op1=AL.add)
            y_all = accp.tile([128, nblk], F32, tag="y_all")
            nc.vector.tensor_scalar(
                out=y_all[:], in0=y1_all[:], scalar1=aux3_s[:, 2:3],
                scalar2=None, op0=AL.add)
            nc.sync.dma_start(out=yd, in_=y_all[:])
    nc.compile()
    return nc, R


def prep_host(f_x, neighbours, seg_ids, f_W1, f_b1, f_W2, f_b2, g_W, g_b):
    """Shard/pack inputs. Returns (Ls, nblk, in_maps, order)."""
    fp8 = ml_dtypes.float8_e4m3
    bf16 = ml_dtypes.bfloat16
    lens_all = np.bincount(seg_ids, minlength=B).astype(np.int64)
    order = np.argsort(-lens_all, kind="stable")
    nblk = B // (SEGS_PER_BLOCK * NCORES)  # 16
    stratum = SEGS_PER_BLOCK * NCORES  # 1024
    Ls = []
    for k in range(nblk):
        m = int(lens_all[order[k * stratum:(k + 1) * stratum]].max())
        Ls.append(max(1, m))
    sumL = sum(Ls)
    R = 128 * sumL

    row_start = np.zeros(B + 1, np.int64)
    row_start[1:] = np.cumsum(lens_all)

    # Householder reflection taking g_n to -s0*|g|*e0: a pure change of the
    # neighbour-feature basis, inverted exactly on the weight side.
    g_n = g_W[0, 1:].astype(np.float32)
    gnorm = float(np.linalg.norm(g_n))
    u = g_n / gnorm
    s0 = 1.0 if u[0] >= 0 else -1.0
    v = u.copy()
    v[0] += s0
    beta = np.float32(2.0 / float(v @ v))
    nf32 = neighbours.astype(np.float32, copy=False)
    x8 = np.empty((T, NF), fp8)
    wbf = np.empty(T, bf16)
    blk = 1 << 18
    for i in range(0, T, blk):
        nb = nf32[i:i + blk]
        q = nb @ v
        rot = nb - (beta * q)[:, None] * v[None, :]
        x8[i:i + blk] = rot.astype(fp8)
        wbf[i:i + blk] = ((-s0 * gnorm) * rot[:, 0]).astype(bf16)

    W1n = f_W1[:, 1:].astype(np.float32)
    W1p = W1n - np.outer(W1n @ v, v) * beta
    w12 = np.ascontiguousarray(W1p.T.astype(bf16))        # [128, 12]

    w1x = f_W1[:, 0].astype(np.float32)
    w2rep = np.tile(f_W2[0].astype(np.float32), max(Ls))
    w2rep = np.tile(w2rep[None, :], (128, 1)).astype(bf16)

    aux3 = np.zeros((128, 3), np.float32)
    aux3[:, 0] = 0.5 * f_b2[0]
    aux3[:, 1] = g_W[0, 0]
    aux3[:, 2] = g_b[0]

    wb = np.empty((128, 2 * H), np.float32)
    wb[:, 0:H] = w1x[None, :]
    wb[:, H:2 * H] = f_b1.astype(np.float32)[None, :]

    wbrep = np.empty((2, H * CH), np.float32)
    wbrep[0] = np.tile(w1x, CH)
    wbrep[1] = np.tile(f_b1.astype(np.float32), CH)
    wbrep = wbrep.astype(bf16)

    in_maps = []
    for c in range(NCORES):
        idx = np.empty(R, np.int64)
        valid = np.empty(R, bool)
        fx_mat = np.empty((128, nblk), np.float32)
        fxT = np.ones((2, 128 * nblk), np.float32)
        nnp = np.empty((128, nblk), np.float32)
        wr = np.empty((128, sumL), bf16)
        off = 0
        soff = 0
        for g in range(nblk):
            Lg = Ls[g]
            gids = order[g * stratum + 128 * c: g * stratum + 128 * (c + 1)]
            pos = np.arange(Lg)[:, None]
            rows = row_start[gids][None, :] + pos          # [Lg, 128]
            val = pos < lens_all[gids][None, :]
            blockn = Lg * 128
            rows_c = np.where(val, rows, 0)
            idx[off:off + blockn] = rows_c.reshape(-1)
            valid[off:off + blockn] = val.reshape(-1)
            wg = wbf[rows_c]                               # [Lg, 128]
            wg[~val] = bf16(0)
            wr[:, soff:soff + Lg] = wg.T
            fx_mat[:, g] = f_x[gids, 0]
            fxT[0, g * 128:(g + 1) * 128] = f_x[gids, 0]
            nnp[:, g] = (lens_all[gids] - Lg).astype(np.float32)
            off += blockn
            soff += Lg
        nrows = x8[idx]                                    # [R, 128] fp8
        nrows[~valid] = fp8(0)
        nt_c = np.ascontiguousarray(nrows.T)               # [128, R]
        in_maps.append({
            "nt": nt_c, "w12": w12, "w2rep": w2rep, "aux3": aux3,
            "fx": fx_mat, "wb": wb, "nnp": nnp, "wr": wr,
            "fxT": fxT.astype(bf16), "wbrep": wbrep,
        })
    return Ls, nblk, in_maps, order


def assemble_output(results, order, nblk):
    stratum = SEGS_PER_BLOCK * NCORES
    y_full = np.empty(B, np.float32)
    for c in range(NCORES):
        yc = results[c]["y"]  # [128, nblk]
        for g in range(nblk):
            y_full[order[g * stratum + 128 * c: g * stratum + 128 * (c + 1)]] = yc[:, g]
    return y_full[:, None]


def kernel(**inputs) -> np.ndarray:
    args = {k: np.asarray(v) for k, v in inputs.items()}
    Ls, nblk, in_maps, order = prep_host(
        args["f_x"], args["neighbours"], args["seg_ids"],
        args["f_W1"], args["f_b1"], args["f_W2"], args["f_b2"],
        args["g_W"], args["g_b"])
    key = (tuple(Ls), nblk)
    if key not in _program_cache:
        _program_cache[key] = build_program(Ls, nblk)
    nc, _ = _program_cache[key]
    res = run_bass_kernel_spmd(nc, in_maps, core_ids=list(range(NCORES)))
    return assemble_output(res.results, order, nblk)
